# revision 8
# baseline (speedup 1.0000x reference)
# Multi-head dot-product attention (B=2, T=4096, E=512, H=8, D=64) with
# causal mask and QK-layernorm, distributed over 8 NeuronCores.
#
# Sharding: head-parallel. Core c handles batch b = c//4 and the adjacent
# head pair p = c%4 (heads 2p, 2p+1). Every core runs an IDENTICAL program
# (SPMD requirement); per-core differences are entirely in the data: the
# host permutes the columns of Wq/Wk so the core's two heads occupy
# columns 0..127 (layernorm stats over the full 512 dims are invariant
# under column permutation), slices Wv columns / Wo rows for those heads,
# and each core emits the partial product attn_out_heads @ Wo_heads.
# The host sums the 4 partials per batch.
#
# On-core pipeline (all matmul inputs bf16, PSUM/softmax math fp32):
#   xT loads:   DMA-transpose x (host-cast bf16) straight from DRAM.
#   proj+LN:    k = x@Wk (PSUM fp32), bn_stats/bn_aggr -> mean/var,
#               (k-mu)*rsqrt(var+eps) fused in one tensor_scalar -> bf16.
#               1/sqrt(D) is folded into q's rsqrt (scale=64 trick).
#   kT/qT:      DMA-transpose the first 128 columns (this core's heads).
#   attention:  S^T[tk,tq] per head via PE (K=64, head pair packed at
#               partition 0/64), causal diagonal handled with a single
#               [128,128] triangle bias added on DVE, exp on ACT
#               ([128,1024] PSUM->SBUF, no max subtraction: |S| <= ~8 by
#               LN construction), P^T@V accumulated in PSUM with a ones
#               column appended to V giving the softmax denominators.
#   finalize:   reciprocal of l, broadcast across partitions with a K=1
#               matmul, scale, then partial @ Wo and DMA out.
import numpy as np
import ml_dtypes

B, T, E, H, D = 2, 4096, 512, 8, 64
EPS = 1e-5
P = 128
NT = T // P          # 32 row tiles
NRB = T // 512       # 8 row blocks / query blocks
NEG = -1.0e30

_cache = {}


def _is_tril(mask2d):
    idx = np.arange(T)
    # mask2d[i, j] should be (j <= i)
    expect = idx[None, :] <= idx[:, None]
    return bool(np.array_equal(mask2d, expect))


def _build(causal):
    import concourse.bass as bass
    import concourse.mybir as mybir
    import concourse.tile as tile
    from concourse import bacc

    f32 = mybir.dt.float32
    bf16 = mybir.dt.bfloat16
    Alu = mybir.AluOpType
    Act = mybir.ActivationFunctionType

    nc = bacc.Bacc(None)
    xqT_d = nc.dram_tensor("xqT", [4, P, T], bf16, kind="ExternalInput")
    xkvT_d = nc.dram_tensor("xkvT", [4, P, T], bf16, kind="ExternalInput")
    wq_d = nc.dram_tensor("wq", [E, E], bf16, kind="ExternalInput")
    wk_d = nc.dram_tensor("wk", [E, E], bf16, kind="ExternalInput")
    wv_d = nc.dram_tensor("wv", [E, 128], bf16, kind="ExternalInput")
    wo_d = nc.dram_tensor("wo", [128, E], bf16, kind="ExternalInput")
    tri_d = nc.dram_tensor("tri", [P, P], f32, kind="ExternalInput")
    ident_d = nc.dram_tensor("ident", [P, P], bf16, kind="ExternalInput")
    out_d = nc.dram_tensor("out", [T, E], f32, kind="ExternalOutput")

    with tile.TileContext(nc) as tc:
        from contextlib import ExitStack

        with ExitStack() as ctx:
            singles = ctx.enter_context(tc.tile_pool(name="singles", bufs=1))
            xqt_p = ctx.enter_context(tc.tile_pool(name="xqt", bufs=2))
            xkt_p = ctx.enter_context(tc.tile_pool(name="xkt", bufs=2))
            ksb_p = ctx.enter_context(tc.tile_pool(name="ksb", bufs=3))
            st_p = ctx.enter_context(tc.tile_pool(name="st", bufs=6))
            pt_p = ctx.enter_context(tc.tile_pool(name="pt", bufs=2))
            at_p = ctx.enter_context(tc.tile_pool(name="at", bufs=2))
            rb_p = ctx.enter_context(tc.tile_pool(name="rb", bufs=2))
            osb_p = ctx.enter_context(tc.tile_pool(name="osb", bufs=3))
            sg_p = ctx.enter_context(
                tc.tile_pool(name="sg", bufs=2, space="PSUM")
            )
            acc_p = ctx.enter_context(
                tc.tile_pool(name="acc", bufs=1, space="PSUM")
            )
            cp_p = ctx.enter_context(
                tc.tile_pool(name="cp", bufs=2, space="PSUM")
            )

            # ---- persistent tiles -------------------------------------
            wq_sb = singles.tile([P, 4, E], bf16, tag="wq")
            wk_sb = singles.tile([P, 4, E], bf16, tag="wk")
            wv_sb = singles.tile([P, 4, 128], bf16, tag="wv")
            wo_sb = singles.tile([64, 2, E], bf16, tag="wo")
            tri_sb = singles.tile([P, P], f32, tag="tri")
            ones_sb = singles.tile([65, 64], f32, tag="ones")
            kT_all = singles.tile([P, T], bf16, tag="kT")
            qT_all = singles.tile([P, T], bf16, tag="qT")
            V_all = singles.tile([P, NT, 130], bf16, tag="V")

            for j in range(4):
                nc.sync.dma_start(out=wq_sb[:, j, :], in_=wq_d[128 * j:128 * j + 128, :])
                nc.sync.dma_start(out=wk_sb[:, j, :], in_=wk_d[128 * j:128 * j + 128, :])
                nc.sync.dma_start(out=wv_sb[:, j, :], in_=wv_d[128 * j:128 * j + 128, :])
            for h in (0, 1):
                nc.sync.dma_start(out=wo_sb[:, h, :], in_=wo_d[64 * h:64 * h + 64, :])
            nc.sync.dma_start(out=tri_sb, in_=tri_d[:, :])
            ident_sb = singles.tile([P, P], bf16, tag="ident")
            nc.sync.dma_start(out=ident_sb, in_=ident_d[:, :])
            nc.gpsimd.memset(ones_sb, 1.0)
            nc.gpsimd.memset(V_all[:, :, 64:65], 1.0)
            nc.gpsimd.memset(V_all[:, :, 129:130], 1.0)
            eps_sb = singles.tile([P, 1], f32, tag="eps")
            eps64_sb = singles.tile([P, 1], f32, tag="eps64")
            nc.vector.memset(eps_sb, EPS)
            nc.vector.memset(eps64_sb, 64.0 * EPS)

            # ---- per row-tile projection + LN -------------------------
            def kv_tile(i, xTb, m):
                ps = sg_p.tile([P, 1024], f32, tag="sg")
                for j in range(4):
                    nc.tensor.matmul(
                        ps[:, 0:512], xTb[:, j, 128 * m:128 * m + 128],
                        wk_sb[:, j, :], start=(j == 0), stop=(j == 3))
                for j in range(4):
                    nc.tensor.matmul(
                        ps[:, 512:640], xTb[:, j, 128 * m:128 * m + 128],
                        wv_sb[:, j, :], start=(j == 0), stop=(j == 3))
                st6 = st_p.tile([P, 6], f32, tag="st6")
                nc.vector.bn_stats(st6, ps[:, 0:512])
                mv = st_p.tile([P, 2], f32, tag="mv")
                nc.vector.bn_aggr(mv, st6)
                std = st_p.tile([P, 1], f32, tag="std")
                nc.scalar.activation(std, mv[:, 1:2], Act.Sqrt, bias=eps_sb,
                                     scale=1.0)
                r = st_p.tile([P, 1], f32, tag="r")
                nc.vector.reciprocal(r, std)
                ksb = ksb_p.tile([P, P], bf16, tag="ksb")
                nc.vector.tensor_scalar(
                    ksb, ps[:, 0:128], mv[:, 0:1], r, Alu.subtract, Alu.mult)
                nc.vector.tensor_copy(V_all[:, i, 0:64], ps[:, 512:576])
                nc.vector.tensor_copy(V_all[:, i, 65:129], ps[:, 576:640])
                tp = cp_p.tile([P, P], bf16, tag="cp")
                nc.tensor.transpose(tp, ksb, ident_sb)
                nc.vector.tensor_copy(kT_all[:, 128 * i:128 * i + 128], tp)

            def q_tile(i, xTb, m):
                ps = sg_p.tile([P, 1024], f32, tag="sg")
                for j in range(4):
                    nc.tensor.matmul(
                        ps[:, 0:512], xTb[:, j, 128 * m:128 * m + 128],
                        wq_sb[:, j, :], start=(j == 0), stop=(j == 3))
                st6 = st_p.tile([P, 6], f32, tag="st6")
                nc.vector.bn_stats(st6, ps[:, 0:512])
                mv = st_p.tile([P, 2], f32, tag="mv")
                nc.vector.bn_aggr(mv, st6)
                std = st_p.tile([P, 1], f32, tag="std")
                # sqrt(64*var + 64*eps) = 8*sqrt(var+eps): folds the 1/sqrt(D)
                # score scale into q's normalization.
                nc.scalar.activation(std, mv[:, 1:2], Act.Sqrt,
                                     bias=eps64_sb, scale=64.0)
                r8 = st_p.tile([P, 1], f32, tag="r")
                nc.vector.reciprocal(r8, std)
                qsb = ksb_p.tile([P, P], bf16, tag="ksb")
                nc.vector.tensor_scalar(
                    qsb, ps[:, 0:128], mv[:, 0:1], r8, Alu.subtract, Alu.mult)
                tp = cp_p.tile([P, P], bf16, tag="cp")
                nc.tensor.transpose(tp, qsb, ident_sb)
                nc.vector.tensor_copy(qT_all[:, 128 * i:128 * i + 128], tp)

            # ---- attention for one 512-row query block ----------------
            def attention(qb):
                acc = acc_p.tile([P, 1024], f32, tag="acc")
                ntk = 4 * qb + 4 if causal else NT
                for tk in range(ntk):
                    sg = sg_p.tile([P, 1024], f32, tag="sg")
                    for h in (0, 1):
                        nc.tensor.matmul(
                            sg[:, 512 * h:512 * h + 512],
                            kT_all[64 * h:64 * h + 64, 128 * tk:128 * tk + 128],
                            qT_all[64 * h:64 * h + 64, 512 * qb:512 * qb + 512],
                            start=True, stop=True)
                    j = tk - 4 * qb
                    diag = causal and j >= 0
                    if diag:
                        for h in (0, 1):
                            lo = 512 * h + 128 * j
                            nc.vector.tensor_add(
                                sg[:, lo:lo + 128], sg[:, lo:lo + 128], tri_sb)
                    pt = pt_p.tile([P, 1024], bf16, tag="pt")
                    nc.scalar.activation(pt, sg[:, :], Act.Exp)
                    for h in (0, 1):
                        lo = 128 * j if (diag and j > 0) else 0
                        nc.tensor.matmul(
                            acc[0:65, 512 * h + lo:512 * h + 512],
                            V_all[:, tk, 65 * h:65 * h + 65],
                            pt[:, 512 * h + lo:512 * h + 512],
                            start=(tk == 0), stop=(tk == ntk - 1))

                # finalize: divide by l, apply Wo, stream out
                rt = st_p.tile([65, 1024], f32, tag="rt")
                nc.vector.reciprocal(rt[64:65, :], acc[64:65, :])
                rb_sb = rb_p.tile([64, 1024], f32, tag="rbs")
                for h in (0, 1):
                    rbps = cp_p.tile([64, 512], f32, tag="cp")
                    nc.tensor.matmul(
                        rbps, ones_sb[64:65, 0:64],
                        rt[64:65, 512 * h:512 * h + 512], start=True, stop=True)
                    nc.vector.tensor_copy(rb_sb[:, 512 * h:512 * h + 512], rbps)
                atn0 = at_p.tile([64, E], bf16, tag="at0")
                atn1 = at_p.tile([64, E], bf16, tag="at1")
                for h, atn in ((0, atn0), (1, atn1)):
                    nc.vector.tensor_mul(
                        atn, acc[0:64, 512 * h:512 * h + 512],
                        rb_sb[0:64, 512 * h:512 * h + 512])
                for m in range(4):
                    ops = cp_p.tile([P, 512], f32, tag="cp")
                    nc.tensor.matmul(ops, atn0[:, 128 * m:128 * m + 128],
                                     wo_sb[:, 0, :], start=True, stop=False)
                    nc.tensor.matmul(ops, atn1[:, 128 * m:128 * m + 128],
                                     wo_sb[:, 1, :], start=False, stop=True)
                    osb = osb_p.tile([P, 512], f32, tag="osb")
                    nc.vector.tensor_copy(osb, ops)
                    row = 512 * qb + 128 * m
                    nc.sync.dma_start(out=out_d[row:row + 128, :], in_=osb)

            # ---- main emission order (software pipeline) --------------
            for rb in range(NRB):
                xqTb = xqt_p.tile([P, 4, 512], bf16, tag="xqt")
                xkTb = xkt_p.tile([P, 4, 512], bf16, tag="xkt")
                for j in range(4):
                    nc.sync.dma_start(
                        out=xqTb[:, j, :],
                        in_=xqT_d[j, :, 512 * rb:512 * rb + 512])
                    nc.sync.dma_start(
                        out=xkTb[:, j, :],
                        in_=xkvT_d[j, :, 512 * rb:512 * rb + 512])
                for m in range(4):
                    i = 4 * rb + m
                    kv_tile(i, xkTb, m)
                    q_tile(i, xqTb, m)
                if causal:
                    attention(rb)
            if not causal:
                for qb in range(NRB):
                    attention(qb)

    if not nc.is_finalized():
        nc.finalize()
    return nc


def _numpy_fallback(inputs_q, inputs_kv, mask, Wq, Wk, Wv, Wo,
                    q_ln_w, q_ln_b, k_ln_w, k_ln_b):
    def ln(x, w, b):
        mu = x.mean(-1, keepdims=True)
        var = ((x - mu) ** 2).mean(-1, keepdims=True)
        return (x - mu) / np.sqrt(var + EPS) * w + b

    q = ln(inputs_q @ Wq, q_ln_w, q_ln_b)
    k = ln(inputs_kv @ Wk, k_ln_w, k_ln_b)
    v = inputs_kv @ Wv
    q = q.reshape(B, T, H, D).transpose(0, 2, 1, 3)
    k = k.reshape(B, T, H, D).transpose(0, 2, 1, 3)
    v = v.reshape(B, T, H, D).transpose(0, 2, 1, 3)
    out = np.empty((B, H, T, D), np.float32)
    m = np.broadcast_to(mask, (B, H, T, T))
    for b in range(B):
        for h in range(H):
            s = (q[b, h] @ k[b, h].T) / np.sqrt(np.float32(D))
            s = np.where(m[b, h], s, -np.inf)
            s -= s.max(-1, keepdims=True)
            p = np.exp(s)
            p /= p.sum(-1, keepdims=True)
            out[b, h] = p @ v[b, h]
    out = out.transpose(0, 2, 1, 3).reshape(B, T, H * D)
    return (out @ Wo).astype(np.float32)


# test harness hooks (ignored by the grader's plain kernel(**inputs) call)
TRACE = False
LAST_RESULTS = None


def kernel(inputs_q, inputs_kv, mask, Wq, Wk, Wv, Wo,
           q_ln_w, q_ln_b, k_ln_w, k_ln_b):
    global LAST_RESULTS
    inputs_q = np.asarray(inputs_q, np.float32)
    inputs_kv = np.asarray(inputs_kv, np.float32)
    mask2d = np.asarray(mask).reshape(mask.shape[-2], mask.shape[-1])
    Wq = np.asarray(Wq, np.float32)
    Wk = np.asarray(Wk, np.float32)
    Wv = np.asarray(Wv, np.float32)
    Wo = np.asarray(Wo, np.float32)

    trivial_ln = (np.all(np.asarray(q_ln_w) == 1) and np.all(np.asarray(q_ln_b) == 0)
                  and np.all(np.asarray(k_ln_w) == 1) and np.all(np.asarray(k_ln_b) == 0))
    causal = _is_tril(mask2d)
    allones = bool(mask2d.all())
    if not trivial_ln or not (causal or allones):
        return _numpy_fallback(inputs_q, inputs_kv, np.asarray(mask), Wq, Wk,
                               Wv, Wo, np.asarray(q_ln_w), np.asarray(q_ln_b),
                               np.asarray(k_ln_w), np.asarray(k_ln_b))

    from concourse.bass_utils import run_bass_kernel_spmd

    key = bool(causal)
    if key not in _cache:
        _cache[key] = _build(causal)
    nc = _cache[key]

    bf = ml_dtypes.bfloat16
    # [4, 128, T] with element [j, d, t] = x[t, 128j + d]
    xT_batches = [
        [np.ascontiguousarray(
            x[b].astype(bf).reshape(T, 4, P).transpose(1, 2, 0))
         for b in range(B)]
        for x in (inputs_q, inputs_kv)]
    ident = np.eye(P, dtype=bf)
    tri = np.where(np.arange(P)[:, None] <= np.arange(P)[None, :],
                   np.float32(0.0), np.float32(NEG))

    in_maps = []
    for c in range(8):
        b, p = c // 4, c % 4
        cols = list(range(128 * p, 128 * p + 128)) + \
            [j for j in range(E) if not (128 * p <= j < 128 * p + 128)]
        in_maps.append(dict(
            xqT=xT_batches[0][b],
            xkvT=xT_batches[1][b],
            wq=Wq[:, cols].astype(bf),
            wk=Wk[:, cols].astype(bf),
            wv=Wv[:, 128 * p:128 * p + 128].astype(bf),
            wo=Wo[128 * p:128 * p + 128, :].astype(bf),
            tri=tri,
            ident=ident,
        ))

    res = run_bass_kernel_spmd(nc, in_maps, list(range(8)), trace=TRACE)
    LAST_RESULTS = res
    outs = [np.asarray(res.results[c]["out"], np.float32) for c in range(8)]
    full = np.stack([outs[0] + outs[1] + outs[2] + outs[3],
                     outs[4] + outs[5] + outs[6] + outs[7]])
    return full


# revision 11
# speedup vs baseline: 1.0719x; 1.0719x over previous
# Multi-head dot-product attention (B=2, T=4096, E=512, H=8, D=64) with
# causal mask and QK-layernorm, distributed over 8 NeuronCores.
#
# Sharding: head-parallel. Core c handles batch b = c//4 and the adjacent
# head pair p = c%4 (heads 2p, 2p+1). Every core runs an IDENTICAL program
# (SPMD requirement); per-core differences are entirely in the data: the
# host permutes the columns of Wq/Wk so the core's two heads occupy
# columns 0..127 (layernorm stats over the full 512 dims are invariant
# under column permutation), slices Wv columns / Wo rows for those heads,
# and each core emits the partial product attn_out_heads @ Wo_heads.
# The host sums the 4 partials per batch.
#
# On-core pipeline (all matmul inputs bf16, PSUM/softmax math fp32):
#   xT loads:   DMA-transpose x (host-cast bf16) straight from DRAM.
#   proj+LN:    k = x@Wk (PSUM fp32), bn_stats/bn_aggr -> mean/var,
#               (k-mu)*rsqrt(var+eps) fused in one tensor_scalar -> bf16.
#               1/sqrt(D) is folded into q's rsqrt (scale=64 trick).
#   kT/qT:      DMA-transpose the first 128 columns (this core's heads).
#   attention:  S^T[tk,tq] per head via PE (K=64, head pair packed at
#               partition 0/64), causal diagonal handled with a single
#               [128,128] triangle bias added on DVE, exp on ACT
#               ([128,1024] PSUM->SBUF, no max subtraction: |S| <= ~8 by
#               LN construction), P^T@V accumulated in PSUM with a ones
#               column appended to V giving the softmax denominators.
#   finalize:   reciprocal of l, broadcast across partitions with a K=1
#               matmul, scale, then partial @ Wo and DMA out.
import numpy as np
import ml_dtypes

B, T, E, H, D = 2, 4096, 512, 8, 64
EPS = 1e-5
P = 128
NT = T // P          # 32 row tiles
NRB = T // 512       # 8 row blocks / query blocks
NEG = -1.0e30

_cache = {}


def _is_tril(mask2d):
    idx = np.arange(T)
    # mask2d[i, j] should be (j <= i)
    expect = idx[None, :] <= idx[:, None]
    return bool(np.array_equal(mask2d, expect))


def _build(causal):
    import concourse.bass as bass
    import concourse.mybir as mybir
    import concourse.tile as tile
    from concourse import bacc

    f32 = mybir.dt.float32
    bf16 = mybir.dt.bfloat16
    Alu = mybir.AluOpType
    Act = mybir.ActivationFunctionType

    nc = bacc.Bacc(None)
    xqT_d = nc.dram_tensor("xqT", [4, P, T], bf16, kind="ExternalInput")
    xkvT_d = nc.dram_tensor("xkvT", [4, P, T], bf16, kind="ExternalInput")
    wq_d = nc.dram_tensor("wq", [E, E], bf16, kind="ExternalInput")
    wk_d = nc.dram_tensor("wk", [E, E], bf16, kind="ExternalInput")
    wv_d = nc.dram_tensor("wv", [E, 128], bf16, kind="ExternalInput")
    wo_d = nc.dram_tensor("wo", [128, E], bf16, kind="ExternalInput")
    tri_d = nc.dram_tensor("tri", [P, P], f32, kind="ExternalInput")
    ident_d = nc.dram_tensor("ident", [P, P], bf16, kind="ExternalInput")
    out_d = nc.dram_tensor("out", [T, E], f32, kind="ExternalOutput")

    with tile.TileContext(nc) as tc:
        from contextlib import ExitStack

        with ExitStack() as ctx:
            singles = ctx.enter_context(tc.tile_pool(name="singles", bufs=1))
            xqt_p = ctx.enter_context(tc.tile_pool(name="xqt", bufs=2))
            xkt_p = ctx.enter_context(tc.tile_pool(name="xkt", bufs=2))
            ksb_p = ctx.enter_context(tc.tile_pool(name="ksb", bufs=3))
            st_p = ctx.enter_context(tc.tile_pool(name="st", bufs=6))
            pt_p = ctx.enter_context(tc.tile_pool(name="pt", bufs=2))
            at_p = ctx.enter_context(tc.tile_pool(name="at", bufs=2))
            rb_p = ctx.enter_context(tc.tile_pool(name="rb", bufs=2))
            osb_p = ctx.enter_context(tc.tile_pool(name="osb", bufs=3))
            sg_p = ctx.enter_context(
                tc.tile_pool(name="sg", bufs=2, space="PSUM")
            )
            acc_p = ctx.enter_context(
                tc.tile_pool(name="acc", bufs=1, space="PSUM")
            )
            cp_p = ctx.enter_context(
                tc.tile_pool(name="cp", bufs=2, space="PSUM")
            )

            # ---- persistent tiles -------------------------------------
            wq_sb = singles.tile([P, 4, E], bf16, tag="wq")
            wk_sb = singles.tile([P, 4, E], bf16, tag="wk")
            wv_sb = singles.tile([P, 4, 128], bf16, tag="wv")
            wo_sb = singles.tile([64, 2, E], bf16, tag="wo")
            tri_sb = singles.tile([P, P], f32, tag="tri")
            ones_sb = singles.tile([65, 64], f32, tag="ones")
            kT_all = singles.tile([P, T], bf16, tag="kT")
            qT_all = singles.tile([P, T], bf16, tag="qT")
            V_all = singles.tile([P, NT, 130], bf16, tag="V")

            for j in range(4):
                nc.sync.dma_start(out=wq_sb[:, j, :], in_=wq_d[128 * j:128 * j + 128, :])
                nc.sync.dma_start(out=wk_sb[:, j, :], in_=wk_d[128 * j:128 * j + 128, :])
                nc.sync.dma_start(out=wv_sb[:, j, :], in_=wv_d[128 * j:128 * j + 128, :])
            for h in (0, 1):
                nc.sync.dma_start(out=wo_sb[:, h, :], in_=wo_d[64 * h:64 * h + 64, :])
            nc.sync.dma_start(out=tri_sb, in_=tri_d[:, :])
            ident_sb = singles.tile([P, P], bf16, tag="ident")
            nc.sync.dma_start(out=ident_sb, in_=ident_d[:, :])
            nc.gpsimd.memset(ones_sb, 1.0)
            nc.gpsimd.memset(V_all[:, :, 64:65], 1.0)
            nc.gpsimd.memset(V_all[:, :, 129:130], 1.0)
            eps_sb = singles.tile([P, 1], f32, tag="eps")
            eps64_sb = singles.tile([P, 1], f32, tag="eps64")
            nc.vector.memset(eps_sb, EPS)
            nc.vector.memset(eps64_sb, 64.0 * EPS)

            # ---- per row-tile projection + LN -------------------------
            def kv_tile(i, xTb, m):
                ps = sg_p.tile([P, 1024], f32, tag="sg")
                for j in range(4):
                    nc.tensor.matmul(
                        ps[:, 0:512], xTb[:, j, 128 * m:128 * m + 128],
                        wk_sb[:, j, :], start=(j == 0), stop=(j == 3))
                for j in range(4):
                    nc.tensor.matmul(
                        ps[:, 512:640], xTb[:, j, 128 * m:128 * m + 128],
                        wv_sb[:, j, :], start=(j == 0), stop=(j == 3))
                st6 = st_p.tile([P, 6], f32, tag="st6")
                nc.vector.bn_stats(st6, ps[:, 0:512])
                mv = st_p.tile([P, 2], f32, tag="mv")
                nc.vector.bn_aggr(mv, st6)
                std = st_p.tile([P, 1], f32, tag="std")
                nc.scalar.activation(std, mv[:, 1:2], Act.Sqrt, bias=eps_sb,
                                     scale=1.0)
                r = st_p.tile([P, 1], f32, tag="r")
                nc.vector.reciprocal_approx_fast(out=r, in_=std)
                ksb = ksb_p.tile([P, P], bf16, tag="ksb")
                nc.vector.tensor_scalar(
                    ksb, ps[:, 0:128], mv[:, 0:1], r, Alu.subtract, Alu.mult)
                nc.vector.tensor_copy(V_all[:, i, 0:64], ps[:, 512:576])
                nc.vector.tensor_copy(V_all[:, i, 65:129], ps[:, 576:640])
                tp = cp_p.tile([P, P], bf16, tag="cp")
                nc.tensor.transpose(tp, ksb, ident_sb)
                nc.vector.tensor_copy(kT_all[:, 128 * i:128 * i + 128], tp)

            def q_tile(i, xTb, m):
                ps = sg_p.tile([P, 1024], f32, tag="sg")
                for j in range(4):
                    nc.tensor.matmul(
                        ps[:, 0:512], xTb[:, j, 128 * m:128 * m + 128],
                        wq_sb[:, j, :], start=(j == 0), stop=(j == 3))
                st6 = st_p.tile([P, 6], f32, tag="st6")
                nc.vector.bn_stats(st6, ps[:, 0:512])
                mv = st_p.tile([P, 2], f32, tag="mv")
                nc.vector.bn_aggr(mv, st6)
                std = st_p.tile([P, 1], f32, tag="std")
                # sqrt(64*var + 64*eps) = 8*sqrt(var+eps): folds the 1/sqrt(D)
                # score scale into q's normalization.
                nc.scalar.activation(std, mv[:, 1:2], Act.Sqrt,
                                     bias=eps64_sb, scale=64.0)
                r8 = st_p.tile([P, 1], f32, tag="r")
                nc.vector.reciprocal_approx_fast(out=r8, in_=std)
                qsb = ksb_p.tile([P, P], bf16, tag="ksb")
                nc.vector.tensor_scalar(
                    qsb, ps[:, 0:128], mv[:, 0:1], r8, Alu.subtract, Alu.mult)
                tp = cp_p.tile([P, P], bf16, tag="cp")
                nc.tensor.transpose(tp, qsb, ident_sb)
                nc.vector.tensor_copy(qT_all[:, 128 * i:128 * i + 128], tp)

            # ---- attention for one 512-row query block ----------------
            def attention(qb):
                acc = acc_p.tile([P, 1024], f32, tag="acc")
                ntk = 4 * qb + 4 if causal else NT
                for tk in range(ntk):
                    sg = sg_p.tile([P, 1024], f32, tag="sg")
                    for h in (0, 1):
                        nc.tensor.matmul(
                            sg[:, 512 * h:512 * h + 512],
                            kT_all[64 * h:64 * h + 64, 128 * tk:128 * tk + 128],
                            qT_all[64 * h:64 * h + 64, 512 * qb:512 * qb + 512],
                            start=True, stop=True)
                    j = tk - 4 * qb
                    diag = causal and j >= 0
                    if diag:
                        for h in (0, 1):
                            lo = 512 * h + 128 * j
                            nc.vector.tensor_add(
                                sg[:, lo:lo + 128], sg[:, lo:lo + 128], tri_sb)
                    pt = pt_p.tile([P, 1024], bf16, tag="pt")
                    nc.scalar.activation(pt, sg[:, :], Act.Exp)
                    for h in (0, 1):
                        lo = 128 * j if (diag and j > 0) else 0
                        nc.tensor.matmul(
                            acc[0:65, 512 * h + lo:512 * h + 512],
                            V_all[:, tk, 65 * h:65 * h + 65],
                            pt[:, 512 * h + lo:512 * h + 512],
                            start=(tk == 0), stop=(tk == ntk - 1))

                # finalize: divide by l, apply Wo, stream out
                lsb = st_p.tile([65, 1024], f32, tag="lsb")
                nc.vector.tensor_copy(lsb[64:65, :], acc[64:65, :])
                l0 = st_p.tile([1, 1024], f32, tag="l0")
                nc.sync.dma_start(out=l0, in_=lsb[64:65, :])
                rt = st_p.tile([1, 1024], f32, tag="rt")
                nc.vector.reciprocal_approx_fast(out=rt, in_=l0)
                rb_sb = rb_p.tile([64, 1024], f32, tag="rbs")
                for h in (0, 1):
                    rbps = cp_p.tile([64, 512], f32, tag="cp")
                    nc.tensor.matmul(
                        rbps, ones_sb[0:1, 0:64],
                        rt[0:1, 512 * h:512 * h + 512], start=True, stop=True)
                    nc.vector.tensor_copy(rb_sb[:, 512 * h:512 * h + 512], rbps)
                atn0 = at_p.tile([64, E], bf16, tag="at0")
                atn1 = at_p.tile([64, E], bf16, tag="at1")
                for h, atn in ((0, atn0), (1, atn1)):
                    nc.vector.tensor_mul(
                        atn, acc[0:64, 512 * h:512 * h + 512],
                        rb_sb[0:64, 512 * h:512 * h + 512])
                for m in range(4):
                    ops = cp_p.tile([P, 512], f32, tag="cp")
                    nc.tensor.matmul(ops, atn0[:, 128 * m:128 * m + 128],
                                     wo_sb[:, 0, :], start=True, stop=False)
                    nc.tensor.matmul(ops, atn1[:, 128 * m:128 * m + 128],
                                     wo_sb[:, 1, :], start=False, stop=True)
                    osb = osb_p.tile([P, 512], f32, tag="osb")
                    nc.vector.tensor_copy(osb, ops)
                    row = 512 * qb + 128 * m
                    nc.sync.dma_start(out=out_d[row:row + 128, :], in_=osb)

            # ---- main emission order (software pipeline) --------------
            for rb in range(NRB):
                xqTb = xqt_p.tile([P, 4, 512], bf16, tag="xqt")
                xkTb = xkt_p.tile([P, 4, 512], bf16, tag="xkt")
                for j in range(4):
                    nc.sync.dma_start(
                        out=xqTb[:, j, :],
                        in_=xqT_d[j, :, 512 * rb:512 * rb + 512])
                    nc.sync.dma_start(
                        out=xkTb[:, j, :],
                        in_=xkvT_d[j, :, 512 * rb:512 * rb + 512])
                for m in range(4):
                    i = 4 * rb + m
                    kv_tile(i, xkTb, m)
                    q_tile(i, xqTb, m)
                if causal:
                    attention(rb)
            if not causal:
                for qb in range(NRB):
                    attention(qb)

    if not nc.is_finalized():
        nc.finalize()
    return nc


def _numpy_fallback(inputs_q, inputs_kv, mask, Wq, Wk, Wv, Wo,
                    q_ln_w, q_ln_b, k_ln_w, k_ln_b):
    def ln(x, w, b):
        mu = x.mean(-1, keepdims=True)
        var = ((x - mu) ** 2).mean(-1, keepdims=True)
        return (x - mu) / np.sqrt(var + EPS) * w + b

    q = ln(inputs_q @ Wq, q_ln_w, q_ln_b)
    k = ln(inputs_kv @ Wk, k_ln_w, k_ln_b)
    v = inputs_kv @ Wv
    q = q.reshape(B, T, H, D).transpose(0, 2, 1, 3)
    k = k.reshape(B, T, H, D).transpose(0, 2, 1, 3)
    v = v.reshape(B, T, H, D).transpose(0, 2, 1, 3)
    out = np.empty((B, H, T, D), np.float32)
    m = np.broadcast_to(mask, (B, H, T, T))
    for b in range(B):
        for h in range(H):
            s = (q[b, h] @ k[b, h].T) / np.sqrt(np.float32(D))
            s = np.where(m[b, h], s, -np.inf)
            s -= s.max(-1, keepdims=True)
            p = np.exp(s)
            p /= p.sum(-1, keepdims=True)
            out[b, h] = p @ v[b, h]
    out = out.transpose(0, 2, 1, 3).reshape(B, T, H * D)
    return (out @ Wo).astype(np.float32)


# test harness hooks (ignored by the grader's plain kernel(**inputs) call)
TRACE = False
LAST_RESULTS = None


def kernel(inputs_q, inputs_kv, mask, Wq, Wk, Wv, Wo,
           q_ln_w, q_ln_b, k_ln_w, k_ln_b):
    global LAST_RESULTS
    inputs_q = np.asarray(inputs_q, np.float32)
    inputs_kv = np.asarray(inputs_kv, np.float32)
    mask2d = np.asarray(mask).reshape(mask.shape[-2], mask.shape[-1])
    Wq = np.asarray(Wq, np.float32)
    Wk = np.asarray(Wk, np.float32)
    Wv = np.asarray(Wv, np.float32)
    Wo = np.asarray(Wo, np.float32)

    trivial_ln = (np.all(np.asarray(q_ln_w) == 1) and np.all(np.asarray(q_ln_b) == 0)
                  and np.all(np.asarray(k_ln_w) == 1) and np.all(np.asarray(k_ln_b) == 0))
    causal = _is_tril(mask2d)
    allones = bool(mask2d.all())
    if not trivial_ln or not (causal or allones):
        return _numpy_fallback(inputs_q, inputs_kv, np.asarray(mask), Wq, Wk,
                               Wv, Wo, np.asarray(q_ln_w), np.asarray(q_ln_b),
                               np.asarray(k_ln_w), np.asarray(k_ln_b))

    from concourse.bass_utils import run_bass_kernel_spmd

    key = bool(causal)
    if key not in _cache:
        _cache[key] = _build(causal)
    nc = _cache[key]

    bf = ml_dtypes.bfloat16
    # [4, 128, T] with element [j, d, t] = x[t, 128j + d]
    xT_batches = [
        [np.ascontiguousarray(
            x[b].astype(bf).reshape(T, 4, P).transpose(1, 2, 0))
         for b in range(B)]
        for x in (inputs_q, inputs_kv)]
    ident = np.eye(P, dtype=bf)
    tri = np.where(np.arange(P)[:, None] <= np.arange(P)[None, :],
                   np.float32(0.0), np.float32(NEG))

    in_maps = []
    for c in range(8):
        b, p = c // 4, c % 4
        cols = list(range(128 * p, 128 * p + 128)) + \
            [j for j in range(E) if not (128 * p <= j < 128 * p + 128)]
        in_maps.append(dict(
            xqT=xT_batches[0][b],
            xkvT=xT_batches[1][b],
            wq=Wq[:, cols].astype(bf),
            wk=Wk[:, cols].astype(bf),
            wv=Wv[:, 128 * p:128 * p + 128].astype(bf),
            wo=Wo[128 * p:128 * p + 128, :].astype(bf),
            tri=tri,
            ident=ident,
        ))

    res = run_bass_kernel_spmd(nc, in_maps, list(range(8)), trace=TRACE)
    LAST_RESULTS = res
    outs = [np.asarray(res.results[c]["out"], np.float32) for c in range(8)]
    full = np.stack([outs[0] + outs[1] + outs[2] + outs[3],
                     outs[4] + outs[5] + outs[6] + outs[7]])
    return full


# revision 20
# speedup vs baseline: 1.0772x; 1.0050x over previous
# Multi-head dot-product attention (B=2, T=4096, E=512, H=8, D=64) with
# causal mask and QK-layernorm, distributed over 8 NeuronCores.
#
# Sharding: head-parallel. Core c handles batch b = c//4 and the adjacent
# head pair p = c%4 (heads 2p, 2p+1). Every core runs an IDENTICAL program
# (SPMD requirement); per-core differences are entirely in the data: the
# host permutes the columns of Wq/Wk so the core's two heads occupy
# columns 0..127 (layernorm stats over the full 512 dims are invariant
# under column permutation), slices Wv columns / Wo rows for those heads,
# and each core emits the partial product attn_out_heads @ Wo_heads.
# The host sums the 4 partials per batch.
#
# On-core pipeline (all matmul inputs bf16, PSUM/softmax math fp32):
#   xT loads:   plain DMA of host-pre-transposed bf16 x.
#   proj+LN:    k = x@Wk (PSUM fp32), bn_stats/bn_aggr -> mean/var,
#               (k-mu)*rsqrt(var+eps) fused in one tensor_scalar -> bf16
#               (only the core's 128 head-columns are materialized).
#               1/sqrt(D) is folded into q's rsqrt (scale=64 trick).
#   kT/qT:      PE transpose (identity matmul) + DVE copy.
#   attention:  S^T[tk,tq] per head via PE (K=64, head pair packed at
#               partition 0/64), causal diagonal handled with a single
#               [128,128] triangle bias added on DVE, exp on ACT
#               ([128,1024] PSUM->SBUF, no max subtraction: |S| <= ~8 by
#               LN construction), P^T@V accumulated in PSUM with a ones
#               column appended to V giving the softmax denominators.
#   finalize:   approx reciprocal of l at partition 0, broadcast across
#               partitions with K=1 matmuls, scale, then the partial @ Wo
#               and DMA out.
import numpy as np
import ml_dtypes

B, T, E, H, D = 2, 4096, 512, 8, 64
EPS = 1e-5
P = 128
NT = T // P          # 32 row tiles
NRB = T // 512       # 8 row blocks / query blocks
NEG = -1.0e30

_cache = {}


def _is_tril(mask2d):
    idx = np.arange(T)
    expect = idx[None, :] <= idx[:, None]
    return bool(np.array_equal(mask2d, expect))


def _build(causal):
    import concourse.bass as bass
    import concourse.mybir as mybir
    import concourse.tile as tile
    from concourse import bacc
    from contextlib import ExitStack

    f32 = mybir.dt.float32
    bf16 = mybir.dt.bfloat16
    Alu = mybir.AluOpType
    Act = mybir.ActivationFunctionType

    nc = bacc.Bacc(None)
    xqT_d = nc.dram_tensor("xqT", [4, P, T], bf16, kind="ExternalInput")
    xkvT_d = nc.dram_tensor("xkvT", [4, P, T], bf16, kind="ExternalInput")
    wq_d = nc.dram_tensor("wq", [E, E], bf16, kind="ExternalInput")
    wk_d = nc.dram_tensor("wk", [E, E], bf16, kind="ExternalInput")
    wv_d = nc.dram_tensor("wv", [E, 128], bf16, kind="ExternalInput")
    wo_d = nc.dram_tensor("wo", [128, E], bf16, kind="ExternalInput")
    tri_d = nc.dram_tensor("tri", [P, P], f32, kind="ExternalInput")
    ident_d = nc.dram_tensor("ident", [P, P], bf16, kind="ExternalInput")
    out_d = nc.dram_tensor("out", [T, E], f32, kind="ExternalOutput")

    with tile.TileContext(nc) as tc:
        with ExitStack() as ctx:
            singles = ctx.enter_context(tc.tile_pool(name="singles", bufs=1))
            xqt_p = ctx.enter_context(tc.tile_pool(name="xqt", bufs=2))
            xkt_p = ctx.enter_context(tc.tile_pool(name="xkt", bufs=2))
            ksb_p = ctx.enter_context(tc.tile_pool(name="ksb", bufs=3))
            st_p = ctx.enter_context(tc.tile_pool(name="st", bufs=6))
            pt_p = ctx.enter_context(tc.tile_pool(name="pt", bufs=2))
            at_p = ctx.enter_context(tc.tile_pool(name="at", bufs=2))
            rb_p = ctx.enter_context(tc.tile_pool(name="rb", bufs=2))
            osb_p = ctx.enter_context(tc.tile_pool(name="osb", bufs=3))
            sg_p = ctx.enter_context(
                tc.tile_pool(name="sg", bufs=2, space="PSUM"))
            acc_p = ctx.enter_context(
                tc.tile_pool(name="acc", bufs=1, space="PSUM"))
            cp_p = ctx.enter_context(
                tc.tile_pool(name="cp", bufs=2, space="PSUM"))

            # ---- persistent tiles -------------------------------------
            wq_sb = singles.tile([P, 4, E], bf16, tag="wq")
            wk_sb = singles.tile([P, 4, E], bf16, tag="wk")
            wv_sb = singles.tile([P, 4, 128], bf16, tag="wv")
            wo_sb = singles.tile([64, 2, E], bf16, tag="wo")
            tri_sb = singles.tile([P, P], f32, tag="tri")
            ident_sb = singles.tile([P, P], bf16, tag="ident")
            ones_sb = singles.tile([65, 64], f32, tag="ones")
            eps_sb = singles.tile([P, 1], f32, tag="eps")
            eps64_sb = singles.tile([P, 1], f32, tag="eps64")
            kT_all = singles.tile([P, T], bf16, tag="kT")
            qT_all = singles.tile([P, T], bf16, tag="qT")
            V_all = singles.tile([P, NT, 130], bf16, tag="V")

            for j in range(4):
                nc.sync.dma_start(out=wq_sb[:, j, :],
                                  in_=wq_d[128 * j:128 * j + 128, :])
                nc.sync.dma_start(out=wk_sb[:, j, :],
                                  in_=wk_d[128 * j:128 * j + 128, :])
                nc.sync.dma_start(out=wv_sb[:, j, :],
                                  in_=wv_d[128 * j:128 * j + 128, :])
            for h in (0, 1):
                nc.sync.dma_start(out=wo_sb[:, h, :],
                                  in_=wo_d[64 * h:64 * h + 64, :])
            nc.sync.dma_start(out=tri_sb, in_=tri_d[:, :])
            nc.sync.dma_start(out=ident_sb, in_=ident_d[:, :])
            nc.gpsimd.memset(ones_sb, 1.0)
            nc.gpsimd.memset(V_all[:, :, 64:65], 1.0)
            nc.gpsimd.memset(V_all[:, :, 129:130], 1.0)
            nc.vector.memset(eps_sb, EPS)
            nc.vector.memset(eps64_sb, 64.0 * EPS)

            # ---- per row-tile projection + LN -------------------------
            def kv_tile(i, xTb, m):
                ps = sg_p.tile([P, 1024], f32, tag="sg")
                for j in range(4):
                    nc.tensor.matmul(
                        ps[:, 0:512], xTb[:, j, 128 * m:128 * m + 128],
                        wk_sb[:, j, :], start=(j == 0), stop=(j == 3))
                for j in range(4):
                    nc.tensor.matmul(
                        ps[:, 512:640], xTb[:, j, 128 * m:128 * m + 128],
                        wv_sb[:, j, :], start=(j == 0), stop=(j == 3))
                st6 = st_p.tile([P, 6], f32, tag="st6")
                nc.vector.bn_stats(st6, ps[:, 0:512])
                mv = st_p.tile([P, 2], f32, tag="mv")
                nc.vector.bn_aggr(mv, st6)
                std = st_p.tile([P, 1], f32, tag="std")
                nc.scalar.activation(std, mv[:, 1:2], Act.Sqrt, bias=eps_sb,
                                     scale=1.0)
                r = st_p.tile([P, 1], f32, tag="r")
                nc.vector.reciprocal_approx_fast(out=r, in_=std)
                ksb = ksb_p.tile([P, P], bf16, tag="ksb")
                nc.vector.tensor_scalar(
                    ksb, ps[:, 0:128], mv[:, 0:1], r, Alu.subtract, Alu.mult)
                nc.vector.tensor_copy(V_all[:, i, 0:64], ps[:, 512:576])
                nc.vector.tensor_copy(V_all[:, i, 65:129], ps[:, 576:640])
                tp = cp_p.tile([P, P], bf16, tag="cp")
                nc.tensor.transpose(tp, ksb, ident_sb)
                nc.vector.tensor_copy(kT_all[:, 128 * i:128 * i + 128], tp)

            def q_tile(i, xTb, m):
                ps = sg_p.tile([P, 1024], f32, tag="sg")
                for j in range(4):
                    nc.tensor.matmul(
                        ps[:, 0:512], xTb[:, j, 128 * m:128 * m + 128],
                        wq_sb[:, j, :], start=(j == 0), stop=(j == 3))
                st6 = st_p.tile([P, 6], f32, tag="st6")
                nc.vector.bn_stats(st6, ps[:, 0:512])
                mv = st_p.tile([P, 2], f32, tag="mv")
                nc.vector.bn_aggr(mv, st6)
                std = st_p.tile([P, 1], f32, tag="std")
                # sqrt(64*var + 64*eps) = 8*sqrt(var+eps): folds the
                # 1/sqrt(D) score scale into q's normalization.
                nc.scalar.activation(std, mv[:, 1:2], Act.Sqrt,
                                     bias=eps64_sb, scale=64.0)
                r8 = st_p.tile([P, 1], f32, tag="r")
                nc.vector.reciprocal_approx_fast(out=r8, in_=std)
                qsb = ksb_p.tile([P, P], bf16, tag="ksb")
                nc.vector.tensor_scalar(
                    qsb, ps[:, 0:128], mv[:, 0:1], r8, Alu.subtract, Alu.mult)
                tp = cp_p.tile([P, P], bf16, tag="cp")
                nc.tensor.transpose(tp, qsb, ident_sb)
                nc.vector.tensor_copy(qT_all[:, 128 * i:128 * i + 128], tp)

            # ---- attention for one 512-row query block ----------------
            def attention(qb):
                acc = acc_p.tile([P, 1024], f32, tag="acc")
                ntk = 4 * qb + 4 if causal else NT
                for tk in range(ntk):
                    sg = sg_p.tile([P, 1024], f32, tag="sg")
                    for h in (0, 1):
                        nc.tensor.matmul(
                            sg[:, 512 * h:512 * h + 512],
                            kT_all[64 * h:64 * h + 64,
                                   128 * tk:128 * tk + 128],
                            qT_all[64 * h:64 * h + 64,
                                   512 * qb:512 * qb + 512],
                            start=True, stop=True)
                    j = tk - 4 * qb
                    diag = causal and j >= 0
                    if diag:
                        for h in (0, 1):
                            lo = 512 * h + 128 * j
                            nc.vector.tensor_add(
                                sg[:, lo:lo + 128], sg[:, lo:lo + 128],
                                tri_sb)
                    pt = pt_p.tile([P, 1024], bf16, tag="pt")
                    nc.scalar.activation(pt, sg[:, :], Act.Exp)
                    for h in (0, 1):
                        lo = 128 * j if (diag and j > 0) else 0
                        nc.tensor.matmul(
                            acc[0:65, 512 * h + lo:512 * h + 512],
                            V_all[:, tk, 65 * h:65 * h + 65],
                            pt[:, 512 * h + lo:512 * h + 512],
                            start=(tk == 0), stop=(tk == ntk - 1))

                # finalize: divide by l, apply Wo, stream out
                lsb = st_p.tile([65, 1024], f32, tag="lsb")
                nc.vector.tensor_copy(lsb[64:65, :], acc[64:65, :])
                l0 = st_p.tile([1, 1024], f32, tag="l0")
                nc.sync.dma_start(out=l0, in_=lsb[64:65, :])
                rt = st_p.tile([1, 1024], f32, tag="rt")
                nc.vector.reciprocal_approx_fast(out=rt, in_=l0)
                rb_sb = rb_p.tile([64, 1024], f32, tag="rbs")
                for h in (0, 1):
                    rbps = cp_p.tile([64, 512], f32, tag="cp")
                    nc.tensor.matmul(
                        rbps, ones_sb[0:1, 0:64],
                        rt[0:1, 512 * h:512 * h + 512], start=True, stop=True)
                    nc.vector.tensor_copy(rb_sb[:, 512 * h:512 * h + 512],
                                          rbps)
                atn0 = at_p.tile([64, E], bf16, tag="at0")
                atn1 = at_p.tile([64, E], bf16, tag="at1")
                for h, atn in ((0, atn0), (1, atn1)):
                    nc.vector.tensor_mul(
                        atn, acc[0:64, 512 * h:512 * h + 512],
                        rb_sb[0:64, 512 * h:512 * h + 512])
                for m in range(4):
                    ops = cp_p.tile([P, 512], f32, tag="cp")
                    nc.tensor.matmul(ops, atn0[:, 128 * m:128 * m + 128],
                                     wo_sb[:, 0, :], start=True, stop=False)
                    nc.tensor.matmul(ops, atn1[:, 128 * m:128 * m + 128],
                                     wo_sb[:, 1, :], start=False, stop=True)
                    osb = osb_p.tile([P, 512], f32, tag="osb")
                    nc.vector.tensor_copy(osb, ops)
                    row = 512 * qb + 128 * m
                    nc.sync.dma_start(out=out_d[row:row + 128, :], in_=osb)

            # ---- main emission order (software pipeline) --------------
            for rb in range(NRB):
                xqTb = xqt_p.tile([P, 4, 512], bf16, tag="xqt")
                xkTb = xkt_p.tile([P, 4, 512], bf16, tag="xkt")
                for j in range(4):
                    nc.sync.dma_start(
                        out=xqTb[:, j, :],
                        in_=xqT_d[j, :, 512 * rb:512 * rb + 512])
                    nc.sync.dma_start(
                        out=xkTb[:, j, :],
                        in_=xkvT_d[j, :, 512 * rb:512 * rb + 512])
                for m in range(4):
                    i = 4 * rb + m
                    kv_tile(i, xkTb, m)
                    q_tile(i, xqTb, m)
                if causal:
                    attention(rb)
            if not causal:
                for qb in range(NRB):
                    attention(qb)

    if not nc.is_finalized():
        nc.finalize()
    return nc


def _numpy_fallback(inputs_q, inputs_kv, mask, Wq, Wk, Wv, Wo,
                    q_ln_w, q_ln_b, k_ln_w, k_ln_b):
    def ln(x, w, b):
        mu = x.mean(-1, keepdims=True)
        var = ((x - mu) ** 2).mean(-1, keepdims=True)
        return (x - mu) / np.sqrt(var + EPS) * w + b

    q = ln(inputs_q @ Wq, q_ln_w, q_ln_b)
    k = ln(inputs_kv @ Wk, k_ln_w, k_ln_b)
    v = inputs_kv @ Wv
    q = q.reshape(B, T, H, D).transpose(0, 2, 1, 3)
    k = k.reshape(B, T, H, D).transpose(0, 2, 1, 3)
    v = v.reshape(B, T, H, D).transpose(0, 2, 1, 3)
    out = np.empty((B, H, T, D), np.float32)
    m = np.broadcast_to(mask, (B, H, T, T))
    for b in range(B):
        for h in range(H):
            s = (q[b, h] @ k[b, h].T) / np.sqrt(np.float32(D))
            s = np.where(m[b, h], s, -np.inf)
            s -= s.max(-1, keepdims=True)
            pr = np.exp(s)
            pr /= pr.sum(-1, keepdims=True)
            out[b, h] = pr @ v[b, h]
    out = out.transpose(0, 2, 1, 3).reshape(B, T, H * D)
    return (out @ Wo).astype(np.float32)


# test harness hooks (ignored by the grader's plain kernel(**inputs) call)
TRACE = False
LAST_RESULTS = None


def kernel(inputs_q, inputs_kv, mask, Wq, Wk, Wv, Wo,
           q_ln_w, q_ln_b, k_ln_w, k_ln_b):
    global LAST_RESULTS
    inputs_q = np.asarray(inputs_q, np.float32)
    inputs_kv = np.asarray(inputs_kv, np.float32)
    mask2d = np.asarray(mask).reshape(mask.shape[-2], mask.shape[-1])
    Wq = np.asarray(Wq, np.float32)
    Wk = np.asarray(Wk, np.float32)
    Wv = np.asarray(Wv, np.float32)
    Wo = np.asarray(Wo, np.float32)

    trivial_ln = (np.all(np.asarray(q_ln_w) == 1)
                  and np.all(np.asarray(q_ln_b) == 0)
                  and np.all(np.asarray(k_ln_w) == 1)
                  and np.all(np.asarray(k_ln_b) == 0))
    causal = _is_tril(mask2d)
    allones = bool(mask2d.all())
    if not trivial_ln or not (causal or allones):
        return _numpy_fallback(inputs_q, inputs_kv, np.asarray(mask), Wq, Wk,
                               Wv, Wo, np.asarray(q_ln_w), np.asarray(q_ln_b),
                               np.asarray(k_ln_w), np.asarray(k_ln_b))

    from concourse.bass_utils import run_bass_kernel_spmd

    key = bool(causal)
    if key not in _cache:
        _cache[key] = _build(causal)
    nc = _cache[key]

    bf = ml_dtypes.bfloat16
    # [4, 128, T] per batch: element [j, d, t] = x[t, 128 j + d]
    xT_batches = [
        [np.ascontiguousarray(
            x[b].astype(bf).reshape(T, 4, P).transpose(1, 2, 0))
         for b in range(B)]
        for x in (inputs_q, inputs_kv)]
    ident = np.eye(P, dtype=bf)
    tri = np.where(np.arange(P)[:, None] <= np.arange(P)[None, :],
                   np.float32(0.0), np.float32(NEG))

    in_maps = []
    for c in range(8):
        b, p = c // 4, c % 4
        cols = list(range(128 * p, 128 * p + 128)) + \
            [jj for jj in range(E) if not (128 * p <= jj < 128 * p + 128)]
        in_maps.append(dict(
            xqT=xT_batches[0][b],
            xkvT=xT_batches[1][b],
            wq=Wq[:, cols].astype(bf),
            wk=Wk[:, cols].astype(bf),
            wv=Wv[:, 128 * p:128 * p + 128].astype(bf),
            wo=Wo[128 * p:128 * p + 128, :].astype(bf),
            tri=tri,
            ident=ident,
        ))

    res = run_bass_kernel_spmd(nc, in_maps, list(range(8)), trace=TRACE)
    LAST_RESULTS = res
    outs = [np.asarray(res.results[c]["out"], np.float32) for c in range(8)]
    full = np.stack([outs[0] + outs[1] + outs[2] + outs[3],
                     outs[4] + outs[5] + outs[6] + outs[7]])
    return full


# revision 21
# speedup vs baseline: 1.1040x; 1.0248x over previous
# Multi-head dot-product attention (B=2, T=4096, E=512, H=8, D=64) with
# causal mask and QK-layernorm, distributed over 8 NeuronCores.
#
# Sharding: head-parallel, projection column-sharded. Core c handles batch
# b = c//4 and head pair p = c%4 (heads 2p, 2p+1). Every core runs an
# IDENTICAL program (SPMD); per-core differences live in the data only:
# each core receives Wq/Wk/Wv column slices and the Wo row slice for its
# two heads, projects ONLY those 128 columns for all 4096 rows, and the
# QK-layernorm statistics (which need full 512-dim rows) are obtained by
# an AllReduce of per-core partial sums (sum k, sum k^2, sum q, sum q^2
# per row -- column-partition invariant), 64KB per group of 4 cores.
# Each core emits the partial product attn_out_heads @ Wo_heads; the host
# sums the 4 partials per batch.
#
# On-core pipeline (matmul inputs bf16, PSUM/softmax math fp32):
#   phase 1: project k/v/q column slices from host-pre-transposed x,
#            stage raw k/q in SBUF, accumulate per-row partial stats.
#   phase 2: AllReduce the stats within the 4-core batch group.
#   phase 3: derive mean and rsqrt(var+eps) for all rows at once
#            (1/sqrt(D) folded into q's rsqrt via the scale=64 trick);
#            only 2 ACT Sqrt instructions total so the exp/sqrt ACT
#            table thrash is gone.
#   phase 4: per 512-row block: LN-apply (fused subtract+mult
#            tensor_scalar), PE-transpose kT/qT, then causal attention for
#            that query block: S^T[tk,tq] per head on PE (K=64, head pair
#            at partitions 0/64), triangle bias added on DVE for diagonal
#            blocks, exp on ACT ([128,1024] PSUM->SBUF, no max
#            subtraction: |S| <= ~8 by LN), P^T@V accumulated in PSUM with
#            a ones column in V giving the softmax denominators l;
#            finalize divides by l (approx reciprocal at partition 0 +
#            K=1 broadcast matmuls) and applies the Wo row slice.
import numpy as np
import ml_dtypes

B, T, E, H, D = 2, 4096, 512, 8, 64
EPS = 1e-5
P = 128
NT = T // P          # 32 row tiles
NRB = T // 512       # 8 row blocks / query blocks
NEG = -1.0e30

_cache = {}


def _is_tril(mask2d):
    idx = np.arange(T)
    expect = idx[None, :] <= idx[:, None]
    return bool(np.array_equal(mask2d, expect))


def _build(causal):
    import concourse.bass as bass
    import concourse.mybir as mybir
    import concourse.tile as tile
    from concourse import bacc
    from contextlib import ExitStack

    f32 = mybir.dt.float32
    bf16 = mybir.dt.bfloat16
    Alu = mybir.AluOpType
    Act = mybir.ActivationFunctionType

    nc = bacc.Bacc(None)
    xqT_d = nc.dram_tensor("xqT", [4, P, T], bf16, kind="ExternalInput")
    xkvT_d = nc.dram_tensor("xkvT", [4, P, T], bf16, kind="ExternalInput")
    wq_d = nc.dram_tensor("wq", [E, P], bf16, kind="ExternalInput")
    wk_d = nc.dram_tensor("wk", [E, P], bf16, kind="ExternalInput")
    wv_d = nc.dram_tensor("wv", [E, P], bf16, kind="ExternalInput")
    wo_d = nc.dram_tensor("wo", [P, E], bf16, kind="ExternalInput")
    tri_d = nc.dram_tensor("tri", [P, P], f32, kind="ExternalInput")
    ident_d = nc.dram_tensor("ident", [P, P], bf16, kind="ExternalInput")
    out_d = nc.dram_tensor("out", [T, E], f32, kind="ExternalOutput")

    with tile.TileContext(nc) as tc:
        with ExitStack() as ctx:
            singles = ctx.enter_context(tc.tile_pool(name="singles", bufs=1))
            xqt_p = ctx.enter_context(tc.tile_pool(name="xqt", bufs=2))
            xkt_p = ctx.enter_context(tc.tile_pool(name="xkt", bufs=2))
            ksb_p = ctx.enter_context(tc.tile_pool(name="ksb", bufs=4))
            scr_p = ctx.enter_context(tc.tile_pool(name="scr", bufs=4))
            st_p = ctx.enter_context(tc.tile_pool(name="st", bufs=6))
            pt_p = ctx.enter_context(tc.tile_pool(name="pt", bufs=2))
            at_p = ctx.enter_context(tc.tile_pool(name="at", bufs=2))
            rb_p = ctx.enter_context(tc.tile_pool(name="rb", bufs=2))
            osb_p = ctx.enter_context(tc.tile_pool(name="osb", bufs=3))
            sg_p = ctx.enter_context(
                tc.tile_pool(name="sg", bufs=2, space="PSUM"))
            acc_p = ctx.enter_context(
                tc.tile_pool(name="acc", bufs=1, space="PSUM"))
            cp_p = ctx.enter_context(
                tc.tile_pool(name="cp", bufs=2, space="PSUM"))
            dram_p = ctx.enter_context(
                tc.tile_pool(name="dram", bufs=1, space="DRAM"))

            # ---- persistent tiles -------------------------------------
            wq_sb = singles.tile([P, 4, P], bf16, tag="wq")
            wk_sb = singles.tile([P, 4, P], bf16, tag="wk")
            wv_sb = singles.tile([P, 4, P], bf16, tag="wv")
            wo_sb = singles.tile([64, 2, E], bf16, tag="wo")
            tri_sb = singles.tile([P, P], f32, tag="tri")
            ident_sb = singles.tile([P, P], bf16, tag="ident")
            ones_sb = singles.tile([65, 64], f32, tag="ones")
            eps_sb = singles.tile([P, 1], f32, tag="eps")
            eps64_sb = singles.tile([P, 1], f32, tag="eps64")
            kT_all = singles.tile([P, T], bf16, tag="kT")
            qT_all = singles.tile([P, T], bf16, tag="qT")
            V_all = singles.tile([P, NT, 130], bf16, tag="V")
            K_raw = singles.tile([P, NT, P], bf16, tag="Kraw")
            Q_raw = singles.tile([P, NT, P], bf16, tag="Qraw")
            stats_loc = singles.tile([P, NT, 4], f32, tag="stl")
            stats_all = singles.tile([P, NT, 4], f32, tag="sta")

            for j in range(4):
                nc.sync.dma_start(out=wq_sb[:, j, :],
                                  in_=wq_d[128 * j:128 * j + 128, :])
                nc.sync.dma_start(out=wk_sb[:, j, :],
                                  in_=wk_d[128 * j:128 * j + 128, :])
                nc.sync.dma_start(out=wv_sb[:, j, :],
                                  in_=wv_d[128 * j:128 * j + 128, :])
            for h in (0, 1):
                nc.sync.dma_start(out=wo_sb[:, h, :],
                                  in_=wo_d[64 * h:64 * h + 64, :])
            nc.sync.dma_start(out=tri_sb, in_=tri_d[:, :])
            nc.sync.dma_start(out=ident_sb, in_=ident_d[:, :])
            nc.gpsimd.memset(ones_sb, 1.0)
            nc.gpsimd.memset(V_all[:, :, 64:65], 1.0)
            nc.gpsimd.memset(V_all[:, :, 129:130], 1.0)
            nc.vector.memset(eps_sb, EPS)
            nc.vector.memset(eps64_sb, 64.0 * EPS)

            # ---- phase 1: column-sliced projections + partial stats ---
            for rb in range(NRB):
                xqTb = xqt_p.tile([P, 4, 512], bf16, tag="xqt")
                xkTb = xkt_p.tile([P, 4, 512], bf16, tag="xkt")
                for j in range(4):
                    nc.sync.dma_start(
                        out=xqTb[:, j, :],
                        in_=xqT_d[j, :, 512 * rb:512 * rb + 512])
                    nc.sync.dma_start(
                        out=xkTb[:, j, :],
                        in_=xkvT_d[j, :, 512 * rb:512 * rb + 512])
                for m in range(4):
                    i = 4 * rb + m
                    ps = sg_p.tile([P, 1024], f32, tag="sg")
                    for j in range(4):
                        nc.tensor.matmul(
                            ps[:, 0:128], xkTb[:, j, 128 * m:128 * m + 128],
                            wk_sb[:, j, :], start=(j == 0), stop=(j == 3))
                    for j in range(4):
                        nc.tensor.matmul(
                            ps[:, 128:256], xkTb[:, j, 128 * m:128 * m + 128],
                            wv_sb[:, j, :], start=(j == 0), stop=(j == 3))
                    for j in range(4):
                        nc.tensor.matmul(
                            ps[:, 256:384], xqTb[:, j, 128 * m:128 * m + 128],
                            wq_sb[:, j, :], start=(j == 0), stop=(j == 3))
                    nc.vector.tensor_copy(K_raw[:, i, :], ps[:, 0:128])
                    nc.vector.tensor_copy(V_all[:, i, 0:64], ps[:, 128:192])
                    nc.vector.tensor_copy(V_all[:, i, 65:129], ps[:, 192:256])
                    nc.vector.tensor_copy(Q_raw[:, i, :], ps[:, 256:384])
                    for (lo, off) in ((0, 0), (256, 2)):
                        st6 = st_p.tile([P, 6], f32, tag="st6")
                        nc.vector.bn_stats(st6, ps[:, lo:lo + 128])
                        mv = st_p.tile([P, 2], f32, tag="mv")
                        nc.vector.bn_aggr(mv, st6)
                        t1 = st_p.tile([P, 1], f32, tag="tt1")
                        nc.vector.tensor_scalar_mul(
                            stats_loc[:, i, off:off + 1], mv[:, 0:1], 128.0)
                        nc.vector.tensor_mul(t1, mv[:, 0:1], mv[:, 0:1])
                        nc.vector.tensor_add(t1, t1, mv[:, 1:2])
                        nc.vector.tensor_scalar_mul(
                            stats_loc[:, i, off + 1:off + 2], t1, 128.0)

            # ---- phase 2: AllReduce stats within the batch group ------
            cc_in = dram_p.tile([P, NT * 4], f32, tag="ccin")
            cc_out = dram_p.tile([P, NT * 4], f32, tag="ccout")
            nc.sync.dma_start(
                out=cc_in[:, :],
                in_=stats_loc.rearrange("p a b -> p (a b)"))
            nc.gpsimd.collective_compute(
                "AllReduce",
                Alu.add,
                replica_groups=[[0, 1, 2, 3], [4, 5, 6, 7]],
                ins=[cc_in.opt()],
                outs=[cc_out.opt()],
            )
            nc.sync.dma_start(
                out=stats_all.rearrange("p a b -> p (a b)"),
                in_=cc_out[:, :])

            # ---- phase 3: derive mu and rsqrt(var+eps) for all rows ---
            sum_k = stats_all[:, :, 0:1].rearrange("p a b -> p (a b)")
            ssq_k = stats_all[:, :, 1:2].rearrange("p a b -> p (a b)")
            sum_q = stats_all[:, :, 2:3].rearrange("p a b -> p (a b)")
            ssq_q = stats_all[:, :, 3:4].rearrange("p a b -> p (a b)")
            mu_k = singles.tile([P, NT], f32, tag="muk")
            mu_q = singles.tile([P, NT], f32, tag="muq")
            r_k = singles.tile([P, NT], f32, tag="rk")
            r_q = singles.tile([P, NT], f32, tag="rq")
            inv_e = 1.0 / float(E)
            for (s_sum, s_ssq, mu, r, e_sb, scale) in (
                    (sum_k, ssq_k, mu_k, r_k, eps_sb, 1.0),
                    (sum_q, ssq_q, mu_q, r_q, eps64_sb, 64.0)):
                tmp1 = st_p.tile([P, NT], f32, tag="t1")
                tmp2 = st_p.tile([P, NT], f32, tag="t2")
                tmp3 = st_p.tile([P, NT], f32, tag="t3")
                nc.vector.tensor_scalar_mul(mu, s_sum, inv_e)
                nc.vector.tensor_scalar_mul(tmp1, s_ssq, inv_e)
                nc.vector.tensor_mul(tmp2, mu, mu)
                nc.vector.tensor_sub(tmp1, tmp1, tmp2)
                # sqrt(scale*var + scale*eps); scale=64 folds 1/sqrt(D)
                nc.scalar.activation(tmp3, tmp1, Act.Sqrt, bias=e_sb,
                                     scale=scale)
                nc.vector.reciprocal_approx_fast(out=r, in_=tmp3)

            # ---- LN-apply + transpose one 128-row tile ----------------
            def finish_tile(i, raw, mu, r, dstT):
                xsb = ksb_p.tile([P, P], bf16, tag="ksb")
                nc.vector.tensor_scalar(
                    xsb, raw[:, i, :], mu[:, i:i + 1], r[:, i:i + 1],
                    Alu.subtract, Alu.mult)
                tp = cp_p.tile([P, P], bf16, tag="cp")
                nc.tensor.transpose(tp, xsb, ident_sb)
                nc.vector.tensor_copy(dstT[:, 128 * i:128 * i + 128], tp)

            # ---- attention for one 512-row query block ----------------
            def attention(qb):
                acc = acc_p.tile([P, 1024], f32, tag="acc")
                ntk = 4 * qb + 4 if causal else NT
                for tk in range(ntk):
                    sg = sg_p.tile([P, 1024], f32, tag="sg")
                    for h in (0, 1):
                        nc.tensor.matmul(
                            sg[:, 512 * h:512 * h + 512],
                            kT_all[64 * h:64 * h + 64,
                                   128 * tk:128 * tk + 128],
                            qT_all[64 * h:64 * h + 64,
                                   512 * qb:512 * qb + 512],
                            start=True, stop=True)
                    j = tk - 4 * qb
                    diag = causal and j >= 0
                    if diag:
                        for h in (0, 1):
                            lo = 512 * h + 128 * j
                            nc.vector.tensor_add(
                                sg[:, lo:lo + 128], sg[:, lo:lo + 128],
                                tri_sb)
                    pt = pt_p.tile([P, 1024], bf16, tag="pt")
                    nc.scalar.activation(pt, sg[:, :], Act.Exp)
                    for h in (0, 1):
                        lo = 128 * j if (diag and j > 0) else 0
                        nc.tensor.matmul(
                            acc[0:65, 512 * h + lo:512 * h + 512],
                            V_all[:, tk, 65 * h:65 * h + 65],
                            pt[:, 512 * h + lo:512 * h + 512],
                            start=(tk == 0), stop=(tk == ntk - 1))

                # finalize: divide by l, apply Wo, stream out
                lsb = st_p.tile([65, 1024], f32, tag="lsb")
                nc.vector.tensor_copy(lsb[64:65, :], acc[64:65, :])
                l0 = st_p.tile([1, 1024], f32, tag="l0")
                nc.sync.dma_start(out=l0, in_=lsb[64:65, :])
                rt = st_p.tile([1, 1024], f32, tag="rt")
                nc.vector.reciprocal_approx_fast(out=rt, in_=l0)
                rb_sb = rb_p.tile([64, 1024], f32, tag="rbs")
                for h in (0, 1):
                    rbps = cp_p.tile([64, 512], f32, tag="cp")
                    nc.tensor.matmul(
                        rbps, ones_sb[0:1, 0:64],
                        rt[0:1, 512 * h:512 * h + 512], start=True, stop=True)
                    nc.vector.tensor_copy(rb_sb[:, 512 * h:512 * h + 512],
                                          rbps)
                atn0 = at_p.tile([64, E], bf16, tag="at0")
                atn1 = at_p.tile([64, E], bf16, tag="at1")
                for h, atn in ((0, atn0), (1, atn1)):
                    nc.vector.tensor_mul(
                        atn, acc[0:64, 512 * h:512 * h + 512],
                        rb_sb[0:64, 512 * h:512 * h + 512])
                for m in range(4):
                    ops = cp_p.tile([P, 512], f32, tag="cp")
                    nc.tensor.matmul(ops, atn0[:, 128 * m:128 * m + 128],
                                     wo_sb[:, 0, :], start=True, stop=False)
                    nc.tensor.matmul(ops, atn1[:, 128 * m:128 * m + 128],
                                     wo_sb[:, 1, :], start=False, stop=True)
                    osb = osb_p.tile([P, 512], f32, tag="osb")
                    nc.vector.tensor_copy(osb, ops)
                    row = 512 * qb + 128 * m
                    nc.sync.dma_start(out=out_d[row:row + 128, :], in_=osb)

            # ---- phase 4: LN finish + attention, pipelined ------------
            for rb in range(NRB):
                for m in range(4):
                    i = 4 * rb + m
                    finish_tile(i, K_raw, mu_k, r_k, kT_all)
                    finish_tile(i, Q_raw, mu_q, r_q, qT_all)
                attention(rb)

    if not nc.is_finalized():
        nc.finalize()
    return nc


def _numpy_fallback(inputs_q, inputs_kv, mask, Wq, Wk, Wv, Wo,
                    q_ln_w, q_ln_b, k_ln_w, k_ln_b):
    def ln(x, w, b):
        mu = x.mean(-1, keepdims=True)
        var = ((x - mu) ** 2).mean(-1, keepdims=True)
        return (x - mu) / np.sqrt(var + EPS) * w + b

    q = ln(inputs_q @ Wq, q_ln_w, q_ln_b)
    k = ln(inputs_kv @ Wk, k_ln_w, k_ln_b)
    v = inputs_kv @ Wv
    q = q.reshape(B, T, H, D).transpose(0, 2, 1, 3)
    k = k.reshape(B, T, H, D).transpose(0, 2, 1, 3)
    v = v.reshape(B, T, H, D).transpose(0, 2, 1, 3)
    out = np.empty((B, H, T, D), np.float32)
    m = np.broadcast_to(mask, (B, H, T, T))
    for b in range(B):
        for h in range(H):
            s = (q[b, h] @ k[b, h].T) / np.sqrt(np.float32(D))
            s = np.where(m[b, h], s, -np.inf)
            s -= s.max(-1, keepdims=True)
            pr = np.exp(s)
            pr /= pr.sum(-1, keepdims=True)
            out[b, h] = pr @ v[b, h]
    out = out.transpose(0, 2, 1, 3).reshape(B, T, H * D)
    return (out @ Wo).astype(np.float32)


# test harness hooks (ignored by the grader's plain kernel(**inputs) call)
TRACE = False
LAST_RESULTS = None


def kernel(inputs_q, inputs_kv, mask, Wq, Wk, Wv, Wo,
           q_ln_w, q_ln_b, k_ln_w, k_ln_b):
    global LAST_RESULTS
    inputs_q = np.asarray(inputs_q, np.float32)
    inputs_kv = np.asarray(inputs_kv, np.float32)
    mask2d = np.asarray(mask).reshape(mask.shape[-2], mask.shape[-1])
    Wq = np.asarray(Wq, np.float32)
    Wk = np.asarray(Wk, np.float32)
    Wv = np.asarray(Wv, np.float32)
    Wo = np.asarray(Wo, np.float32)

    trivial_ln = (np.all(np.asarray(q_ln_w) == 1)
                  and np.all(np.asarray(q_ln_b) == 0)
                  and np.all(np.asarray(k_ln_w) == 1)
                  and np.all(np.asarray(k_ln_b) == 0))
    causal = _is_tril(mask2d)
    allones = bool(mask2d.all())
    if not trivial_ln or not (causal or allones):
        return _numpy_fallback(inputs_q, inputs_kv, np.asarray(mask), Wq, Wk,
                               Wv, Wo, np.asarray(q_ln_w), np.asarray(q_ln_b),
                               np.asarray(k_ln_w), np.asarray(k_ln_b))

    from concourse.bass_utils import run_bass_kernel_spmd

    key = bool(causal)
    if key not in _cache:
        _cache[key] = _build(causal)
    nc = _cache[key]

    bf = ml_dtypes.bfloat16
    # [4, 128, T] per batch: element [j, d, t] = x[t, 128 j + d]
    xT_batches = [
        [np.ascontiguousarray(
            x[b].astype(bf).reshape(T, 4, P).transpose(1, 2, 0))
         for b in range(B)]
        for x in (inputs_q, inputs_kv)]
    ident = np.eye(P, dtype=bf)
    tri = np.where(np.arange(P)[:, None] <= np.arange(P)[None, :],
                   np.float32(0.0), np.float32(NEG))

    in_maps = []
    for c in range(8):
        b, p = c // 4, c % 4
        sl = slice(128 * p, 128 * p + 128)
        in_maps.append(dict(
            xqT=xT_batches[0][b],
            xkvT=xT_batches[1][b],
            wq=Wq[:, sl].astype(bf),
            wk=Wk[:, sl].astype(bf),
            wv=Wv[:, sl].astype(bf),
            wo=Wo[sl, :].astype(bf),
            tri=tri,
            ident=ident,
        ))

    res = run_bass_kernel_spmd(nc, in_maps, list(range(8)), trace=TRACE)
    LAST_RESULTS = res
    outs = [np.asarray(res.results[c]["out"], np.float32) for c in range(8)]
    full = np.stack([outs[0] + outs[1] + outs[2] + outs[3],
                     outs[4] + outs[5] + outs[6] + outs[7]])
    return full
